# revision 14
# baseline (speedup 1.0000x reference)
"""Trainium2 Bass kernel for DilatedCausalSelfAttention (B=1, L=4096, E=1024,
16 heads, d=64; branches (w,r) = (1024,1), (2048,2), (4096,4)).

Head-sharded: 2 heads per core (core c owns heads 2c, 2c+1).
- P1(b0): dense Q^T/K^T (d-major bf16) + V pos-major straight into the
  PV-ready Vaug layout (no transposes).
- Branch 1/2 Q/K are strided column-selects of the dense Q/K done on DVE
  (copy + copy_predicated with per-core 0/1 masks shipped as data, so the
  SPMD program stays uniform); only V is recomputed per branch.
- P2: software-pipelined windowed causal attention: score tiles for two
  steps ahead are issued before each PV so the in-order PE queue never
  blocks on the Act exp round-trip; kt4+5 / kt6+7 share one exp. The causal
  mask is folded in as a PE-accumulated NEG upper-triangle matmul.
- Each branch is staged + AllToAll'd immediately after its attention so the
  collectives for branches 0/1 overlap remaining compute. Post-collective
  merge offsets depend only on the head-block index (uniform).
"""

import numpy as np

import concourse.bacc as bacc
import concourse.tile as tile
from concourse import mybir
from concourse.bass import AP
from concourse.bass_utils import run_bass_kernel_spmd

F32 = mybir.dt.float32
BF16 = mybir.dt.bfloat16

N_CORES = 8
L = 4096
E = 1024
D = 64
G = 1024                      # sparse window length (w // r, same for all branches)
KT = 8                        # 128-row key tiles per window
RATIOS = [1, 2, 4]
LBS = [L // r for r in RATIOS]          # per-branch sparse length
VOFF = [0, LBS[0], LBS[0] + LBS[1]]     # offsets into concatenated vpativ
NEG = -30000.0
EXP = mybir.ActivationFunctionType.Exp

# P2 score-tile layout: (width, [(kt, col_offset), ...])
STILES = [
    (1024, [(0, 0)]),
    (896, [(1, 0)]),
    (768, [(2, 0)]),
    (640, [(3, 0)]),
    (896, [(4, 0), (5, 512)]),
    (384, [(6, 0), (7, 256)]),
]


def build_nc():
    nc = bacc.Bacc("TRN2", target_bir_lowering=False, debug=False,
                   num_devices=N_CORES)

    xts = [nc.dram_tensor(f"xt{b}", [E, LBS[b]], BF16, kind="ExternalInput").ap()
           for b in range(3)]
    wq = nc.dram_tensor("wq", [E, 128], BF16, kind="ExternalInput").ap()
    wk = nc.dram_tensor("wk", [E, 128], BF16, kind="ExternalInput").ap()
    wv = nc.dram_tensor("wv", [E, 128], BF16, kind="ExternalInput").ap()
    wproj = nc.dram_tensor("wproj", [E, E], BF16, kind="ExternalInput").ap()
    ident = nc.dram_tensor("ident", [128, 128], BF16, kind="ExternalInput").ap()
    maskneg = nc.dram_tensor("maskneg", [128, 128], BF16, kind="ExternalInput").ap()
    vpativ = nc.dram_tensor("vpativ", [1, sum(LBS)], BF16, kind="ExternalInput").ap()
    # select masks: [m1 (2048) | m2a m2b m2c (3*1024)]
    selm = nc.dram_tensor("selm", [128, 2048 + 3 * 1024], mybir.dt.uint8,
                          kind="ExternalInput").ap()
    out = nc.dram_tensor("out", [512, E], F32, kind="ExternalOutput").ap()

    from contextlib import ExitStack
    with tile.TileContext(nc) as tc, ExitStack() as stk:
        # ---- persistent pools -------------------------------------------------
        consts = stk.enter_context(tc.tile_pool(name="consts", bufs=1))
        xtp = stk.enter_context(tc.tile_pool(name="xtp", bufs=1))
        qkp = stk.enter_context(tc.tile_pool(name="qkp", bufs=1))
        QT = [qkp.tile([128, LBS[b]], BF16, name=f"QT{b}") for b in range(3)]
        KTb = [qkp.tile([128, LBS[b]], BF16, name=f"KT{b}") for b in range(3)]

        # first x block for branch 0 before anything else, then weights
        xt_first = xtp.tile([128, 8, 512], BF16, tag="xt", bufs=2)
        nc.sync.dma_start(
            xt_first[:], xts[0][:, 0:512].rearrange("(k p) c -> p k c", p=128))
        w_sb = {}
        for name, ap in (("q", wq), ("k", wk), ("v", wv)):
            t = consts.tile([128, 8, 128], BF16, name=f"w{name}sb")
            nc.sync.dma_start(t[:], ap.rearrange("(k p) c -> p k c", p=128))
            w_sb[name] = t
        # small consts + select masks + wproj on the Act DMA queue
        ident_sb = consts.tile([128, 128], BF16)
        nc.scalar.dma_start(ident_sb[:], ident[:])
        mask_sb = consts.tile([128, 128], BF16)
        nc.scalar.dma_start(mask_sb[:], maskneg[:])
        viv_sb = consts.tile([65, sum(LBS)], BF16)
        nc.scalar.dma_start(viv_sb[64:65, :], vpativ[0:1, :])
        selm_sb = consts.tile([128, 2048 + 3 * 1024], mybir.dt.uint8)
        nc.scalar.dma_start(selm_sb[:], selm[:])
        wpp = stk.enter_context(tc.tile_pool(name="wpp", bufs=1))
        wp_sb = []
        for jj in range(8):
            t = wpp.tile([128, E], BF16, tag=f"wp{jj}")
            wp_sb.append(t)

        def load_wproj():
            for jj in range(8):
                nc.scalar.dma_start(wp_sb[jj][:],
                                    wproj[128 * jj:128 * (jj + 1), :])

        vaugp = stk.enter_context(tc.tile_pool(name="vaugp", bufs=1))
        # V_aug per branch: tile t block of 130 cols = [h0 V|1][h1 V|1]
        Vaug = [vaugp.tile([128, (LBS[b] // 128) * 130], BF16, name=f"Va{b}")
                for b in range(3)]
        for b in range(3):
            ones3 = Vaug[b][:].rearrange("p (t c) -> p t c", c=65)
            nc.vector.memset(ones3[:, :, 64:65], 1.0)

        ftp = stk.enter_context(tc.tile_pool(name="ftp", bufs=1))
        FT = [ftp.tile([128, LBS[b]], BF16, name=f"FT{b}") for b in range(3)]
        dram = stk.enter_context(tc.tile_pool(name="dram", bufs=1, space="DRAM"))
        a2a_in = [dram.tile([1024, LBS[b] // 8], BF16, name=f"a2ain{b}")
                  for b in range(3)]
        a2a_out = [dram.tile([1024, LBS[b] // 8], BF16, name=f"a2aout{b}")
                   for b in range(3)]

        ptp = stk.enter_context(tc.tile_pool(name="ptp", bufs=1))
        PT = [ptp.tile([128, 512], BF16, tag=f"pt{jj}", name=f"PT{jj}")
              for jj in range(8)]
        T1 = [ptp.tile([128, 256], BF16, tag=f"t1{jj}", name=f"T1{jj}")
              for jj in range(8)]
        T2 = [ptp.tile([128, 128], BF16, tag=f"t2{jj}", name=f"T2{jj}")
              for jj in range(8)]

        esp = stk.enter_context(tc.tile_pool(name="esp", bufs=1))
        epip = stk.enter_context(tc.tile_pool(name="epip", bufs=1))

        def p1(b, qk):
            """V (pos-major into Vaug) and, if qk, dense Q^T/K^T for branch b."""
            with tc.tile_pool(name="qkvps", bufs=1, space="PSUM") as ps:
                nblk = LBS[b] // 512
                for s in range(nblk):
                    if b == 0 and s == 0:
                        xt_t = xt_first
                    else:
                        xt_t = xtp.tile([128, 8, 512], BF16, tag="xt", bufs=2)
                        nc.sync.dma_start(
                            xt_t[:],
                            xts[b][:, 512 * s:512 * (s + 1)]
                            .rearrange("(k p) c -> p k c", p=128))
                    if qk:
                        psq = ps.tile([128, 512], F32, tag="psq", bufs=2)
                        psk = ps.tile([128, 512], F32, tag="psk", bufs=2)
                        for k in range(8):
                            nc.tensor.matmul(psq[:], w_sb["q"][:, k, :],
                                             xt_t[:, k, :], start=(k == 0),
                                             stop=(k == 7))
                        for k in range(8):
                            nc.tensor.matmul(psk[:], w_sb["k"][:, k, :],
                                             xt_t[:, k, :], start=(k == 0),
                                             stop=(k == 7))
                        nc.scalar.copy(QT[b][:, 512 * s:512 * (s + 1)], psq[:])
                        nc.scalar.copy(KTb[b][:, 512 * s:512 * (s + 1)], psk[:])
                    # V pos-major: 4 pos-tiles of 128, dims on free axis
                    psv = ps.tile([128, 512], F32, tag="psv", bufs=2)
                    for t in range(4):
                        for k in range(8):
                            nc.tensor.matmul(
                                psv[:, 128 * t:128 * (t + 1)],
                                xt_t[:, k, 128 * t:128 * (t + 1)],
                                w_sb["v"][:, k, :],
                                start=(k == 0), stop=(k == 7))
                    vdst = Vaug[b][:, 520 * s:520 * (s + 1)].rearrange(
                        "p (t h c) -> p t h c", t=4, c=65)[:, :, :, 0:64]
                    vsrc = psv[:].rearrange("p (t h c) -> p t h c", t=4, c=64)
                    nc.vector.tensor_copy(vdst, vsrc)

        # -- DVE select pieces for branch 1/2 Q/K (issued between epilogues) --
        def select_pieces():
            ps = []
            for dst, src in ((QT[1], QT[0]), (KTb[1], KTb[0])):
                for p in range(4):   # 512-col pieces of the 2048-wide dst
                    sl = slice(512 * p, 512 * (p + 1))
                    s0 = src[:, 1024 * p:1024 * (p + 1):2]
                    s1 = src[:, 1024 * p + 1:1024 * (p + 1):2]
                    m = selm_sb[:, sl]
                    ps.append((dst[:, sl], m, s0, s1, None, None))
            for dst, src in ((QT[2], QT[0]), (KTb[2], KTb[0])):
                for p in range(2):   # 512-col pieces of the 1024-wide dst
                    sl = slice(512 * p, 512 * (p + 1))
                    cls = [src[:, 2048 * p + t:2048 * (p + 1):4] for t in range(4)]
                    ms = [selm_sb[:, 2048 + 1024 * t + sl.start:
                                  2048 + 1024 * t + sl.stop] for t in range(3)]
                    ps.append((dst[:, sl], ms, cls, None, None, "b2"))
            return ps

        def emit_select(piece):
            if piece[-1] == "b2":
                dst, ms, cls = piece[0], piece[1], piece[2]
                nc.vector.tensor_copy(dst, cls[0])
                for t in range(3):
                    nc.vector.copy_predicated(dst, ms[t], cls[t + 1])
            else:
                dst, m, s0, s1 = piece[0], piece[1], piece[2], piece[3]
                nc.vector.tensor_copy(dst, s0)
                nc.vector.copy_predicated(dst, m, s1)

        def p2(b, fillers=()):
            """Windowed causal attention for branch b -> FT[b].

            One global software pipeline across every (window, head, tile):
            score tiles run two steps ahead of PV consumption so the in-order
            PE queue never blocks on the Act exp round-trip.
            fillers: DVE work pieces interleaved after each window epilogue.
            """
            fill = list(fillers)
            with (tc.tile_pool(name="spps", bufs=1, space="PSUM") as spps,
                  tc.tile_pool(name="ops", bufs=1, space="PSUM") as ops):
                nwin = LBS[b] // G
                jobs = [(n, hh, i) for n in range(nwin) for hh in range(2)
                        for i in range(6)]
                state = {}

                def S(job):
                    n, hh, i = job
                    hs = 64 * hh
                    wd, kts = STILES[i]
                    sp = spps.tile([128, G], F32, tag="sp", bufs=3)
                    for kt, off in kts:
                        nq = G - 128 * kt
                        base = G * n + 128 * kt
                        lhsT = KTb[b][hs:hs + 64, base:base + 128]
                        pA = min(nq, 512)
                        nc.tensor.matmul(
                            sp[:, off:off + pA], lhsT,
                            QT[b][hs:hs + 64, base:base + pA],
                            start=True, stop=True)
                        if nq > 512:
                            nc.tensor.matmul(
                                sp[:, 512:nq], lhsT,
                                QT[b][hs:hs + 64, base + 512:base + nq],
                                start=True, stop=True)
                        nc.tensor.matmul(
                            sp[:, off:off + 128], ident_sb[:],
                            mask_sb[:], start=False, stop=True,
                            skip_group_check=True)
                    es = esp.tile([128, G], BF16, tag="es", bufs=4)
                    nc.scalar.activation(es[:, 0:wd], sp[:, 0:wd], EXP)
                    state[job] = es

                def PV(job):
                    n, hh, i = job
                    if i == 0:
                        state[(n, hh)] = ops.tile([65, G], F32, tag="o",
                                                  bufs=1, name="Owh")
                    O = state[(n, hh)]
                    es = state.pop(job)
                    for kt, off in STILES[i][1]:
                        nq = G - 128 * kt
                        va = Vaug[b][:, 130 * (KT * n + kt) + 65 * hh:
                                     130 * (KT * n + kt) + 65 * hh + 65]
                        if kt < 4:
                            pv1 = 512 - 128 * kt
                            nc.tensor.matmul(O[:, 128 * kt:512], va,
                                             es[:, off:off + pv1],
                                             start=(kt == 0), stop=(kt == 3),
                                             skip_group_check=True)
                            nc.tensor.matmul(O[:, 512:G], va,
                                             es[:, off + pv1:off + nq],
                                             start=(kt == 0), stop=(kt == 7),
                                             skip_group_check=True)
                        else:
                            nc.tensor.matmul(O[:, 128 * kt:G], va,
                                             es[:, off:off + nq],
                                             start=False, stop=(kt == 7),
                                             skip_group_check=True)

                def epilogue(n, hh):
                    hs = 64 * hh
                    O = state.pop((n, hh))
                    # single PSUM reader: copy O (incl. den row) to SBUF
                    OS = epip.tile([65, G], BF16, tag="os", bufs=2)
                    nc.vector.tensor_copy(OS[:], O[:])
                    denv = epip.tile([1, G], F32, tag="denv", bufs=2)
                    nc.vector.tensor_mul(
                        denv[:], OS[64:65, :],
                        viv_sb[64:65, VOFF[b] + G * n:VOFF[b] + G * (n + 1)])
                    rcp = epip.tile([1, G], F32, tag="rcp", bufs=2)
                    nc.vector.reciprocal_approx_fast(rcp[:], denv[:])
                    rdd = dram.tile([1, G], F32, tag="rdd", bufs=2, name="rdd")
                    nc.sync.dma_start(rdd[:], rcp[:])
                    sclb = epip.tile([64, G], F32, tag="sclb", bufs=2)
                    rsrc = rdd[:]
                    nc.sync.dma_start(
                        sclb[:],
                        AP(rsrc.tensor, rsrc.offset,
                           [[0, 64]] + list(rsrc.ap)[1:]))
                    nc.vector.tensor_mul(
                        FT[b][hs:hs + 64, G * n:G * (n + 1)],
                        OS[0:64, :], sclb[:])

                S(jobs[0]); S(jobs[1])
                nwh = 2 * nwin
                done = 0
                for t, job in enumerate(jobs):
                    if t + 2 < len(jobs):
                        S(jobs[t + 2])
                    PV(job)
                    if job[2] == 5:
                        epilogue(job[0], job[1])
                        done += 1
                        nsel = max(1, (len(fill) + nwh - done) // (nwh - done)) \
                            if done < nwh else len(fill)
                        for _ in range(nsel):
                            if fill:
                                emit_select(fill.pop(0))
            for piece in fill:
                emit_select(piece)

        def stage_cc(b):
            w = LBS[b] // 8
            for j in range(8):
                nc.gpsimd.dma_start(a2a_in[b][128 * j:128 * (j + 1), :],
                                    FT[b][:, w * j:w * (j + 1)])
            nc.gpsimd.collective_compute(
                "AllToAll", mybir.AluOpType.bypass,
                replica_groups=[list(range(N_CORES))],
                ins=[a2a_in[b].opt()], outs=[a2a_out[b].opt()])

        # ---- phases -----------------------------------------------------------
        p1(0, qk=True)
        p2(0, fillers=select_pieces())
        stage_cc(0)
        load_wproj()
        for jj in range(8):
            nc.gpsimd.dma_start(PT[jj][:], a2a_out[0][128 * jj:128 * (jj + 1), :])
        p1(1, qk=False)
        p2(1)
        stage_cc(1)
        for jj in range(8):
            nc.gpsimd.dma_start(T1[jj][:], a2a_out[1][128 * jj:128 * (jj + 1), :])
        # merge branch 1 into PT (offset depends on head-block jj only)
        for jj in range(8):
            pt2 = PT[jj][:].rearrange("p (t c) -> p t c", c=2)
            i2 = jj // 4
            nc.vector.tensor_add(pt2[:, :, i2:i2 + 1], pt2[:, :, i2:i2 + 1],
                                 T1[jj][:].rearrange("p (t c) -> p t c", c=1))
        p1(2, qk=False)
        p2(2)
        stage_cc(2)
        for jj in range(8):
            nc.gpsimd.dma_start(T2[jj][:], a2a_out[2][128 * jj:128 * (jj + 1), :])
        for jj in range(8):
            pt4 = PT[jj][:].rearrange("p (t c) -> p t c", c=4)
            i4 = jj // 2
            nc.vector.tensor_add(pt4[:, :, i4:i4 + 1], pt4[:, :, i4:i4 + 1],
                                 T2[jj][:].rearrange("p (t c) -> p t c", c=1))

        # ---- projection -------------------------------------------------------
        with (tc.tile_pool(name="prps", bufs=1, space="PSUM") as prps,
              tc.tile_pool(name="ocp", bufs=1) as ocp):
            for m in range(4):
                for nb in range(2):
                    pp = prps.tile([128, 512], F32, tag="pp", bufs=2)
                    for jj in range(8):
                        nc.tensor.matmul(pp[:], PT[jj][:, 128 * m:128 * (m + 1)],
                                         wp_sb[jj][:, 512 * nb:512 * (nb + 1)],
                                         start=(jj == 0), stop=(jj == 7))
                    oc = ocp.tile([128, 512], F32, tag="oc", bufs=2)
                    nc.scalar.copy(oc[:], pp[:])
                    nc.sync.dma_start(out[128 * m:128 * (m + 1),
                                          512 * nb:512 * (nb + 1)], oc[:])
    nc.compile()
    return nc


_NC_CACHE = None


def _get_nc():
    global _NC_CACHE
    if _NC_CACHE is None:
        _NC_CACHE = build_nc()
    return _NC_CACHE


def _host_inputs(x, w_qkv, w_proj):
    import ml_dtypes
    bf = ml_dtypes.bfloat16
    xT = np.ascontiguousarray(x[0].T).astype(np.float32)      # (E, L)
    f = np.arange(128)
    ident = np.eye(128, dtype=np.float32).astype(bf)
    maskneg = np.where(f[:, None] > f[None, :], NEG, 0.0).astype(np.float32).astype(bf)
    in_maps = []
    for c in range(N_CORES):
        h = 2 * c
        vivs = []
        for b, r in enumerate(RATIOS):
            i = h // (16 // r)
            cs = r * np.arange(L // r) + i
            V = 1 + (cs % 2 == h // 8).astype(np.int32) \
                  + (cs % 4 == h // 4).astype(np.int32)
            vivs.append(V.astype(np.float32))
        o1, o2 = h // 8, (h // 4) % 4
        sel = np.zeros((128, 2048 + 3 * 1024), np.uint8)
        sel[:, 0:2048] = o1                              # b1: use odd class?
        for t in range(3):
            sel[:, 2048 + 1024 * t:2048 + 1024 * (t + 1)] = int(o2 == t + 1)
        i2, i4 = c // 4, c // 2
        m = {
            "xt0": xT,
            "xt1": np.ascontiguousarray(xT[:, i2::2]),
            "xt2": np.ascontiguousarray(xT[:, i4::4]),
            "wq": np.ascontiguousarray(w_qkv[:, 128 * c:128 * (c + 1)]) / 8.0,
            "wk": np.ascontiguousarray(w_qkv[:, E + 128 * c:E + 128 * (c + 1)]),
            "wv": np.ascontiguousarray(w_qkv[:, 2 * E + 128 * c:2 * E + 128 * (c + 1)]),
            "wproj": np.ascontiguousarray(w_proj),
            "ident": ident,
            "maskneg": maskneg,
            "vpativ": np.concatenate(vivs)[None, :],
            "selm": sel,
        }
        im = {}
        for k, v in m.items():
            if k in ("ident", "maskneg", "selm"):
                im[k] = np.ascontiguousarray(v)
            else:
                im[k] = np.ascontiguousarray(
                    np.asarray(v, np.float32).astype(bf))
        in_maps.append(im)
    return in_maps


def kernel(x, w_qkv, w_proj, _trace=False):
    x = np.asarray(x, np.float32)
    w_qkv = np.asarray(w_qkv, np.float32)
    w_proj = np.asarray(w_proj, np.float32)
    nc = _get_nc()
    in_maps = _host_inputs(x, w_qkv, w_proj)
    res = run_bass_kernel_spmd(nc, in_maps, core_ids=list(range(N_CORES)),
                               trace=_trace)
    full = np.empty((L, E), np.float32)
    for c in range(N_CORES):
        full[512 * c:512 * (c + 1)] = res.results[c]["out"]
    out = full.reshape(1, L, E)
    if _trace:
        return out, res
    return out
